# revision 21
# baseline (speedup 1.0000x reference)
"""PEER / product-key MoE routing kernel for Trainium2 (8 NeuronCores).

Strategy: data-parallel over tokens. Each of the 8 cores gets 256 of the
2048 tokens plus a full replica of the expert tables in its DRAM. Routing
(q projection, product-key scores, two-stage top-8), expert-row gathers,
and the PEER combine all run on-device. No collectives are needed; the
host only slices/packs inputs and concatenates the per-core outputs.

Per-core pipeline:
  PE:  qT = Wq^T @ x^T (fp32, exact), s1/s2 = qT_half^T @ keysT (fp32)
  DVE: top-8 of each 256-score set via max8/max_index (exact), then
       top-8 of the 8x8 combo sums the same way; winners' sub-key ids
       resolved with an is_equal one-hot reduction (no per-partition
       gather primitive exists on this hardware)
  GPSIMD: indirect-DMA gathers of expert rows. w_down/w_up are packed
       side by side on the host ([65536, 1024] bf16) so one gather per
       (token-block, slot) fetches both rows: 128 partitions x 2KB.
  DVE/ACT/PE: inner products as bf16 multiply (DVE 2x mode) + free-dim
       sum on the scalar engine (activation accum_out); combine as
       PSUM-accumulated diag(vals) @ w_up_row matmuls on PE.

Routing is computed entirely in fp32, so expert selection matches the
fp32 reference exactly; only the expert tables are bf16 (rel err ~4e-3).
HW-measured: ~345 us on 8 cores, vs ~186 us gather-bandwidth roofline.
"""

import numpy as np

import concourse.bass as bass
import concourse.mybir as mybir
from concourse import bacc
from concourse.bass import IndirectOffsetOnAxis
from concourse.tile import TileContext
from concourse.bass_utils import run_bass_kernel_spmd

N_CORES = 8
N_HEADS = 8
D_KEYS = 128
HALF = 64
N_KEYS = 256
TOP_K = 8
D = 512
B = 2048           # total tokens
BC = B // N_CORES  # tokens per core (256)
TB = BC // 128     # token blocks per core (2)
GS = 8             # expert rows gathered per indirect DMA per partition
F32 = mybir.dt.float32
U16 = mybir.dt.uint16
U32 = mybir.dt.uint32
I32 = mybir.dt.int32
BF16 = mybir.dt.bfloat16
X = mybir.AxisListType.X
OP = mybir.AluOpType


def build_nc(stage="full"):
    nc = bacc.Bacc("TRN2", target_bir_lowering=False)

    xtok_d = nc.dram_tensor("xtok", [BC, D], F32, kind="ExternalInput")
    xt_d = nc.dram_tensor("xt", [D, BC], F32, kind="ExternalInput")
    wq_d = nc.dram_tensor("wq", [D, N_HEADS * D_KEYS], F32, kind="ExternalInput")
    bqp_d = nc.dram_tensor("bqp", [HALF, 16], F32, kind="ExternalInput")
    kp1_d = nc.dram_tensor("kp1", [HALF, N_HEADS, N_KEYS], F32, kind="ExternalInput")
    kp2_d = nc.dram_tensor("kp2", [HALF, N_HEADS, N_KEYS], F32, kind="ExternalInput")
    wb_d = nc.dram_tensor("wb", [N_KEYS * N_KEYS, 2 * D], BF16,
                          kind="ExternalInput")
    id01_d = nc.dram_tensor("id01", [128, 128], BF16, kind="ExternalInput")
    out_d = nc.dram_tensor("out", [BC, D], F32, kind="ExternalOutput")

    with TileContext(nc) as tc:
        with (
            tc.tile_pool(name="const", bufs=1) as cpool,
            tc.tile_pool(name="qt", bufs=1) as qtpool,
            tc.tile_pool(name="psq", bufs=2, space="PSUM") as psq,
            tc.tile_pool(name="pss", bufs=2, space="PSUM") as pss,
            tc.tile_pool(name="sc", bufs=4) as scpool,
            tc.tile_pool(name="st2", bufs=1) as st2,
            tc.tile_pool(name="eqs", bufs=2) as eqs,
            tc.tile_pool(name="wbp", bufs=28) as wbp,
            tc.tile_pool(name="ttr", bufs=6) as ttrp,
            tc.tile_pool(name="dgp", bufs=8) as dgp,
            tc.tile_pool(name="pacc", bufs=2, space="PSUM") as paccp,
            tc.tile_pool(name="accp", bufs=2) as accp,
        ):
            # ---- constant loads ----
            wq_sb = []
            xt_sb = []
            for k in range(4):
                t = cpool.tile([128, N_HEADS * D_KEYS], F32, tag=f"wq{k}")
                nc.sync.dma_start(out=t[:], in_=wq_d[k * 128:(k + 1) * 128, :])
                wq_sb.append(t)
                t2 = cpool.tile([128, BC], F32, tag=f"xt{k}")
                nc.sync.dma_start(out=t2[:], in_=xt_d[k * 128:(k + 1) * 128, :])
                xt_sb.append(t2)
            xtok_sb = []
            xtok_bf = []
            for tb in range(TB):
                t = cpool.tile([128, D], F32, tag=f"xtok{tb}")
                nc.sync.dma_start(out=t[:], in_=xtok_d[tb * 128:(tb + 1) * 128, :])
                xtok_sb.append(t)
                tb16 = cpool.tile([128, D], BF16, tag=f"xtokb{tb}")
                nc.vector.tensor_copy(out=tb16[:], in_=t[:])
                xtok_bf.append(tb16)
            kp1_sb = cpool.tile([HALF, N_HEADS, N_KEYS], F32, tag="kp1")
            nc.sync.dma_start(out=kp1_sb[:], in_=kp1_d[:, :, :])
            kp2_sb = cpool.tile([HALF, N_HEADS, N_KEYS], F32, tag="kp2")
            nc.sync.dma_start(out=kp2_sb[:], in_=kp2_d[:, :, :])
            id01_sb = cpool.tile([128, 128], BF16, tag="id01")
            nc.sync.dma_start(out=id01_sb[:], in_=id01_d[:, :])
            bqp_sb = cpool.tile([HALF, 16], F32, tag="bqp")
            nc.sync.dma_start(out=bqp_sb[:], in_=bqp_d[:, :])
            iota8 = cpool.tile([128, 8], U16, tag="iota8")
            nc.gpsimd.iota(iota8[:], pattern=[[1, 8]], base=0, channel_multiplier=0)
            iota64 = cpool.tile([128, 64], U32, tag="iota64")
            nc.gpsimd.iota(iota64[:], pattern=[[1, 64]], base=0, channel_multiplier=0)

            # ---- qT: [feature, token], 16 column-tiles of 64 features ----
            # feature f = m*128 + j*64 + p  ->  column mj = 2*m + j
            qt_all = qtpool.tile([HALF, 16, BC], F32, tag="qt_all")
            for mj in range(16):
                ps = psq.tile([HALF, BC], F32, tag="psq")
                for k in range(4):
                    nc.tensor.matmul(
                        out=ps[:],
                        lhsT=wq_sb[k][:, mj * HALF:(mj + 1) * HALF],
                        rhs=xt_sb[k][:],
                        start=(k == 0),
                        stop=(k == 3),
                    )
                # add per-feature bias (per-partition scalar) and move to SBUF
                nc.vector.tensor_scalar(
                    out=qt_all[:, mj, :], in0=ps[:],
                    scalar1=bqp_sb[:, mj:mj + 1], scalar2=None, op0=OP.add,
                )

            idx32 = []
            v8s = []
            for tb in range(TB):
                tsl = slice(tb * 128, (tb + 1) * 128)
                s1t = st2.tile([128, 64], F32, tag=f"s1t{tb}")
                s2t = st2.tile([128, 64], F32, tag=f"s2t{tb}")
                i1 = st2.tile([128, 64], U16, tag=f"i1{tb}")
                i2 = st2.tile([128, 64], U16, tag=f"i2{tb}")
                # ---- scores + stage-1 top8 (exact) ----
                for m in range(N_HEADS):
                    for half, (kp, st_, ix) in enumerate(
                        ((kp1_sb, s1t, i1), (kp2_sb, s2t, i2))
                    ):
                        ps = pss.tile([128, N_KEYS], F32, tag="pss")
                        nc.tensor.matmul(
                            out=ps[:],
                            lhsT=qt_all[:, 2 * m + half, tsl],
                            rhs=kp[:, m, :],
                            start=True, stop=True,
                        )
                        s_sb = scpool.tile([128, N_KEYS], F32, tag="s_sb")
                        nc.scalar.copy(out=s_sb[:], in_=ps[:])
                        nc.vector.max(out=st_[:, m * 8:(m + 1) * 8], in_=s_sb[:])
                        nc.vector.max_index(
                            out=ix[:, m * 8:(m + 1) * 8],
                            in_max=st_[:, m * 8:(m + 1) * 8],
                            in_values=s_sb[:],
                        )

                # ---- stage-2: 8x8 combo scores, mantissa-encoded top8 ----
                cs = st2.tile([128, 512], F32, tag=f"cs{tb}")
                for m in range(N_HEADS):
                    nc.vector.tensor_tensor(
                        out=cs[:, m * 64:(m + 1) * 64].rearrange(
                            "p (a b) -> p a b", a=8),
                        in0=s1t[:, m * 8:(m + 1) * 8].unsqueeze(2).to_broadcast(
                            [128, 8, 8]),
                        in1=s2t[:, m * 8:(m + 1) * 8].unsqueeze(1).to_broadcast(
                            [128, 8, 8]),
                        op=OP.add,
                    )
                v8 = st2.tile([128, 64], F32, tag=f"v8{tb}")
                n8 = st2.tile([128, 64], U16, tag=f"n8{tb}")
                for m in range(N_HEADS):
                    nc.vector.max(out=v8[:, m * 8:(m + 1) * 8],
                                  in_=cs[:, m * 64:(m + 1) * 64])
                    nc.vector.max_index(
                        out=n8[:, m * 8:(m + 1) * 8],
                        in_max=v8[:, m * 8:(m + 1) * 8],
                        in_values=cs[:, m * 64:(m + 1) * 64])
                k1 = st2.tile([128, 64], U16, tag=f"k1{tb}")
                nc.vector.tensor_scalar(
                    out=k1[:], in0=n8[:], scalar1=3, scalar2=None,
                    op0=OP.logical_shift_right)
                k2 = st2.tile([128, 64], U16, tag=f"k2{tb}")
                nc.vector.tensor_scalar(
                    out=k2[:], in0=n8[:], scalar1=7, scalar2=None,
                    op0=OP.bitwise_and)

                # resolve winners' sub-key ids: isel[p,m,j] = i[p,m,k1[p,m,j]]
                sels = []
                for kk, ix in ((k1, i1), (k2, i2)):
                    eq = eqs.tile([128, 512], U16, tag="eq")
                    nc.vector.tensor_tensor(
                        out=eq[:, :].rearrange("p (m j k) -> p m j k", m=8, j=8),
                        in0=kk[:, :].rearrange("p (m j) -> p m j", m=8)
                            .unsqueeze(3).to_broadcast([128, 8, 8, 8]),
                        in1=iota8[:, :].unsqueeze(1).unsqueeze(1)
                            .to_broadcast([128, 8, 8, 8]),
                        op=OP.is_equal)
                    prod = eqs.tile([128, 512], U16, tag="prod")
                    nc.vector.tensor_tensor(
                        out=prod[:, :].rearrange("p (m j k) -> p m j k", m=8, j=8),
                        in0=eq[:, :].rearrange("p (m j k) -> p m j k", m=8, j=8),
                        in1=ix[:, :].rearrange("p (m k) -> p m k", m=8)
                            .unsqueeze(2).to_broadcast([128, 8, 8, 8]),
                        op=OP.mult)
                    sel = st2.tile([128, 64], U16, tag=f"sel{len(sels)}{tb}")
                    with nc.allow_low_precision(
                            reason="one-hot uint16 sum, values <= 255"):
                        nc.vector.reduce_sum(
                            out=sel[:],
                            in_=prod[:, :].rearrange("p (mj k) -> p mj k", k=8),
                            axis=X)
                    sels.append(sel)
                idx16 = st2.tile([128, 64], U16, tag=f"idx16{tb}")
                nc.vector.tensor_scalar(
                    out=idx16[:], in0=sels[0][:], scalar1=256, scalar2=None,
                    op0=OP.mult)
                nc.vector.tensor_tensor(
                    out=idx16[:], in0=idx16[:], in1=sels[1][:], op=OP.add)
                ix32 = st2.tile([128, 64], I32, tag=f"idx32{tb}")
                nc.vector.tensor_copy(out=ix32[:], in_=idx16[:])
                idx32.append(ix32)
                v8s.append(v8)

            if stage == "routing":
                for tb in range(TB):
                    dbg = st2.tile([128, 64], F32, tag=f"dbg{tb}")
                    nc.vector.tensor_copy(out=dbg[:], in_=idx32[tb][:])
                    nc.sync.dma_start(
                        out=out_d[tb * 128:(tb + 1) * 128, 0:64], in_=dbg[:])
                    nc.sync.dma_start(
                        out=out_d[tb * 128:(tb + 1) * 128, 64:128],
                        in_=v8s[tb][:])

            # ---- softmax over each head's top-8 (on encoded scores) ----
            ws = []
            for tb in (() if stage == "routing" else range(TB)):
                v8 = v8s[tb]
                rmax = st2.tile([128, 8], F32, tag=f"rmax{tb}")
                nc.vector.reduce_max(
                    out=rmax[:], in_=v8[:, :].rearrange("p (m k) -> p m k", m=8),
                    axis=X)
                ex = st2.tile([128, 64], F32, tag=f"ex{tb}")
                nc.vector.tensor_tensor(
                    out=ex[:, :].rearrange("p (m k) -> p m k", m=8),
                    in0=v8[:, :].rearrange("p (m k) -> p m k", m=8),
                    in1=rmax[:, :].unsqueeze(2).to_broadcast([128, 8, 8]),
                    op=OP.subtract)
                nc.scalar.activation(
                    out=ex[:], in_=ex[:], func=mybir.ActivationFunctionType.Exp)
                rsum = st2.tile([128, 8], F32, tag=f"rsum{tb}")
                nc.vector.reduce_sum(
                    out=rsum[:], in_=ex[:, :].rearrange("p (m k) -> p m k", m=8),
                    axis=X)
                rinv = st2.tile([128, 8], F32, tag=f"rinv{tb}")
                nc.vector.reciprocal(out=rinv[:], in_=rsum[:])
                w8 = st2.tile([128, 64], F32, tag=f"w8{tb}")
                nc.vector.tensor_tensor(
                    out=w8[:, :].rearrange("p (m k) -> p m k", m=8),
                    in0=ex[:, :].rearrange("p (m k) -> p m k", m=8),
                    in1=rinv[:, :].unsqueeze(2).to_broadcast([128, 8, 8]),
                    op=OP.mult)
                ws.append(w8)

            # ---- fused main loop: gather both rows, inner, combine ----
            # Token-blocks are interleaved at group granularity so both
            # PSUM accumulation chains (PE) and gather streams (GpSimd)
            # stay busy concurrently.
            tbs = () if stage in ("routing",) else tuple(range(TB))
            inner = {}
            va = {}
            pacc = {}
            pages = {}
            for tb in tbs:
                inner[tb] = st2.tile([128, 64], F32, tag=f"inner{tb}", name=f"inner{tb}")
                va[tb] = st2.tile([128, 64], F32, tag=f"va{tb}", name=f"va{tb}")
                pacc[tb] = paccp.tile([128, D], F32, tag=f"pacc{tb}", name=f"pacc{tb}")
            for grp in range(8):
                for tb in tbs:
                    for sidx in range(8):
                        col = grp * 8 + sidx
                        page = wbp.tile([128, 2 * D], BF16, tag="wbpage",
                                        name=f"pg{tb}_{col}")
                        pages[(tb, col)] = page
                        nc.gpsimd.indirect_dma_start(
                            out=page[:], out_offset=None,
                            in_=wb_d[:, :],
                            in_offset=IndirectOffsetOnAxis(
                                ap=idx32[tb][:, col:col + 1], axis=0),
                        )
                        scr = ttrp.tile([128, D], BF16, tag="ttr_scr")
                        nc.vector.tensor_tensor(
                            out=scr[:], in0=page[:, 0:D], in1=xtok_bf[tb][:],
                            op=OP.mult)
                        if sidx % 2 == 0:
                            scr2 = ttrp.tile([128, D], BF16, tag="ttr_scr2")
                            nc.scalar.activation(
                                out=scr2[:], in_=scr[:],
                                func=mybir.ActivationFunctionType.Copy,
                                accum_out=inner[tb][:, col:col + 1])
                        else:
                            nc.vector.reduce_sum(
                                out=inner[tb][:, col:col + 1], in_=scr[:],
                                axis=X)
                    gs = slice(grp * 8, (grp + 1) * 8)
                    rl8 = st2.tile([128, 8], F32, tag=f"rl{tb}")
                    nc.scalar.activation(
                        out=rl8[:], in_=inner[tb][:, gs],
                        func=mybir.ActivationFunctionType.Relu)
                    if stage == "wdown":
                        nc.vector.tensor_copy(out=va[tb][:, gs], in_=rl8[:])
                    else:
                        nc.vector.tensor_tensor(
                            out=va[tb][:, gs], in0=rl8[:], in1=ws[tb][:, gs],
                            op=OP.mult)
                    for sidx in range(8):
                        col = grp * 8 + sidx
                        diag = dgp.tile([128, 128], BF16, tag="diag")
                        nc.vector.tensor_tensor(
                            out=diag[:],
                            in0=va[tb][:, col:col + 1].to_broadcast([128, 128]),
                            in1=id01_sb[:], op=OP.mult)
                        nc.tensor.matmul(
                            out=pacc[tb][:], lhsT=diag[:],
                            rhs=pages[(tb, col)][:, D:2 * D],
                            start=(col == 0), stop=(col == 63))
            for tb in tbs:
                acc_sb = accp.tile([128, D], F32, tag=f"acc{tb}")
                nc.vector.tensor_copy(out=acc_sb[:], in_=pacc[tb][:])
                if stage == "wdown":
                    nc.sync.dma_start(
                        out=out_d[tb * 128:(tb + 1) * 128, 0:64],
                        in_=inner[tb][:])
                else:
                    nc.sync.dma_start(
                        out=out_d[tb * 128:(tb + 1) * 128, :], in_=acc_sb[:])

    nc.compile()
    return nc


_NC_CACHE = None


def _get_nc():
    global _NC_CACHE
    if _NC_CACHE is None:
        _NC_CACHE = build_nc()
    return _NC_CACHE


def _prep_in_maps(inputs):
    q = np.ascontiguousarray(np.asarray(inputs["queries"], dtype=np.float32))
    Wq = np.ascontiguousarray(np.asarray(inputs["Wq"], dtype=np.float32))
    bq = np.asarray(inputs["bq"], dtype=np.float32)
    keys = np.asarray(inputs["keys"], dtype=np.float32)
    wd = np.asarray(inputs["w_down"], dtype=np.float32)
    wu = np.asarray(inputs["w_up"], dtype=np.float32)
    import ml_dtypes
    wb = np.ascontiguousarray(
        np.concatenate([wd, wu], axis=1).astype(ml_dtypes.bfloat16))
    id01 = np.eye(128, dtype=np.float32).astype(ml_dtypes.bfloat16)

    x = q.reshape(B, D)
    # bqp[p, mj] = bq[mj*64 + p]
    bqp = np.ascontiguousarray(bq.reshape(16, HALF).T)
    # kp{1,2}[c, m, n] = keys[m, half, n, c]
    kp1 = np.ascontiguousarray(keys[:, 0].transpose(2, 0, 1))
    kp2 = np.ascontiguousarray(keys[:, 1].transpose(2, 0, 1))

    in_maps = []
    for c in range(N_CORES):
        xc = x[c * BC:(c + 1) * BC]
        in_maps.append({
            "xtok": np.ascontiguousarray(xc),
            "xt": np.ascontiguousarray(xc.T),
            "wq": Wq,
            "bqp": bqp,
            "kp1": kp1,
            "kp2": kp2,
            "wb": wb,
            "id01": id01,
        })
    return in_maps


def run(inputs, trace=False):
    """Run on 8 NeuronCores; returns (out [2,1024,512], BassKernelResults)."""
    nc = _get_nc()
    in_maps = _prep_in_maps(inputs)
    res = run_bass_kernel_spmd(
        nc, in_maps, core_ids=list(range(N_CORES)), trace=trace)
    out = np.concatenate(
        [res.results[c]["out"] for c in range(N_CORES)], axis=0)
    return out.reshape(2, 1024, D), res


def kernel(**inputs) -> np.ndarray:
    out, _ = run(inputs, trace=False)
    return out


# revision 22
# speedup vs baseline: 1.0260x; 1.0260x over previous
"""PEER / product-key MoE routing kernel for Trainium2 (8 NeuronCores).

Strategy: data-parallel over tokens. Each of the 8 cores gets 256 of the
2048 tokens plus a full replica of the expert tables in its DRAM. Routing
(q projection, product-key scores, two-stage top-8), expert-row gathers,
and the PEER combine all run on-device. No collectives are needed; the
host only slices/packs inputs and concatenates the per-core outputs.

Per-core pipeline:
  PE:  qT = Wq^T @ x^T (fp32, exact), s1/s2 = qT_half^T @ keysT (fp32)
  DVE: top-8 of each 256-score set via max8/max_index (exact), then
       top-8 of the 8x8 combo sums the same way; winners' sub-key ids
       resolved with an is_equal one-hot reduction (no per-partition
       gather primitive exists on this hardware)
  GPSIMD: indirect-DMA gathers of expert rows. w_down/w_up are packed
       side by side on the host ([65536, 1024] bf16) so one gather per
       (token-block, slot) fetches both rows: 128 partitions x 2KB.
  DVE/ACT/PE: inner products as bf16 multiply (DVE 2x mode) + free-dim
       sum on the scalar engine (activation accum_out); combine as
       PSUM-accumulated diag(vals) @ w_up_row matmuls on PE.

Routing is computed entirely in fp32, so expert selection matches the
fp32 reference exactly; only the expert tables are bf16 (rel err ~4e-3).
HW-measured: ~345 us on 8 cores, vs ~186 us gather-bandwidth roofline.
"""

import numpy as np

import concourse.bass as bass
import concourse.mybir as mybir
from concourse import bacc
from concourse.bass import IndirectOffsetOnAxis
from concourse.tile import TileContext
from concourse.bass_utils import run_bass_kernel_spmd

N_CORES = 8
N_HEADS = 8
D_KEYS = 128
HALF = 64
N_KEYS = 256
TOP_K = 8
D = 512
B = 2048           # total tokens
BC = B // N_CORES  # tokens per core (256)
TB = BC // 128     # token blocks per core (2)
GS = 8             # expert rows gathered per indirect DMA per partition
F32 = mybir.dt.float32
U16 = mybir.dt.uint16
U32 = mybir.dt.uint32
I32 = mybir.dt.int32
BF16 = mybir.dt.bfloat16
X = mybir.AxisListType.X
OP = mybir.AluOpType


def build_nc(stage="full"):
    nc = bacc.Bacc("TRN2", target_bir_lowering=False)

    xtok_d = nc.dram_tensor("xtok", [BC, D], F32, kind="ExternalInput")
    xt_d = nc.dram_tensor("xt", [D, BC], F32, kind="ExternalInput")
    wq_d = nc.dram_tensor("wq", [D, N_HEADS * D_KEYS], F32, kind="ExternalInput")
    bqp_d = nc.dram_tensor("bqp", [HALF, 16], F32, kind="ExternalInput")
    kp1_d = nc.dram_tensor("kp1", [HALF, N_HEADS, N_KEYS], F32, kind="ExternalInput")
    kp2_d = nc.dram_tensor("kp2", [HALF, N_HEADS, N_KEYS], F32, kind="ExternalInput")
    wb_d = nc.dram_tensor("wb", [N_KEYS * N_KEYS, 2 * D], BF16,
                          kind="ExternalInput")
    id01_d = nc.dram_tensor("id01", [128, 128], BF16, kind="ExternalInput")
    out_d = nc.dram_tensor("out", [BC, D], F32, kind="ExternalOutput")

    with TileContext(nc) as tc:
        with (
            tc.tile_pool(name="const", bufs=1) as cpool,
            tc.tile_pool(name="qt", bufs=1) as qtpool,
            tc.tile_pool(name="psq", bufs=2, space="PSUM") as psq,
            tc.tile_pool(name="pss", bufs=2, space="PSUM") as pss,
            tc.tile_pool(name="sc", bufs=4) as scpool,
            tc.tile_pool(name="st2", bufs=1) as st2,
            tc.tile_pool(name="eqs", bufs=2) as eqs,
            tc.tile_pool(name="wbp", bufs=28) as wbp,
            tc.tile_pool(name="ttr", bufs=6) as ttrp,
            tc.tile_pool(name="dgp", bufs=8) as dgp,
            tc.tile_pool(name="pacc", bufs=2, space="PSUM") as paccp,
            tc.tile_pool(name="accp", bufs=2) as accp,
        ):
            # ---- constant loads ----
            wq_sb = []
            xt_sb = []
            for k in range(4):
                t = cpool.tile([128, N_HEADS * D_KEYS], F32, tag=f"wq{k}")
                nc.sync.dma_start(out=t[:], in_=wq_d[k * 128:(k + 1) * 128, :])
                wq_sb.append(t)
                t2 = cpool.tile([128, BC], F32, tag=f"xt{k}")
                nc.sync.dma_start(out=t2[:], in_=xt_d[k * 128:(k + 1) * 128, :])
                xt_sb.append(t2)
            xtok_sb = []
            xtok_bf = []
            for tb in range(TB):
                t = cpool.tile([128, D], F32, tag=f"xtok{tb}")
                nc.sync.dma_start(out=t[:], in_=xtok_d[tb * 128:(tb + 1) * 128, :])
                xtok_sb.append(t)
                tb16 = cpool.tile([128, D], BF16, tag=f"xtokb{tb}")
                nc.vector.tensor_copy(out=tb16[:], in_=t[:])
                xtok_bf.append(tb16)
            kp1_sb = cpool.tile([HALF, N_HEADS, N_KEYS], F32, tag="kp1")
            nc.sync.dma_start(out=kp1_sb[:], in_=kp1_d[:, :, :])
            kp2_sb = cpool.tile([HALF, N_HEADS, N_KEYS], F32, tag="kp2")
            nc.sync.dma_start(out=kp2_sb[:], in_=kp2_d[:, :, :])
            id01_sb = cpool.tile([128, 128], BF16, tag="id01")
            nc.sync.dma_start(out=id01_sb[:], in_=id01_d[:, :])
            bqp_sb = cpool.tile([HALF, 16], F32, tag="bqp")
            nc.sync.dma_start(out=bqp_sb[:], in_=bqp_d[:, :])
            iota8 = cpool.tile([128, 8], U16, tag="iota8")
            nc.gpsimd.iota(iota8[:], pattern=[[1, 8]], base=0, channel_multiplier=0)
            iota64 = cpool.tile([128, 64], U32, tag="iota64")
            nc.gpsimd.iota(iota64[:], pattern=[[1, 64]], base=0, channel_multiplier=0)

            # ---- qT: [feature, token], 16 column-tiles of 64 features ----
            # feature f = m*128 + j*64 + p  ->  column mj = 2*m + j
            qt_all = qtpool.tile([HALF, 16, BC], F32, tag="qt_all")
            for mj in range(16):
                ps = psq.tile([HALF, BC], F32, tag="psq")
                for k in range(4):
                    nc.tensor.matmul(
                        out=ps[:],
                        lhsT=wq_sb[k][:, mj * HALF:(mj + 1) * HALF],
                        rhs=xt_sb[k][:],
                        start=(k == 0),
                        stop=(k == 3),
                    )
                # add per-feature bias (per-partition scalar) and move to SBUF
                nc.vector.tensor_scalar(
                    out=qt_all[:, mj, :], in0=ps[:],
                    scalar1=bqp_sb[:, mj:mj + 1], scalar2=None, op0=OP.add,
                )

            idx32 = []
            v8s = []
            for tb in range(TB):
                tsl = slice(tb * 128, (tb + 1) * 128)
                s1t = st2.tile([128, 64], F32, tag=f"s1t{tb}")
                s2t = st2.tile([128, 64], F32, tag=f"s2t{tb}")
                i1 = st2.tile([128, 64], U16, tag=f"i1{tb}")
                i2 = st2.tile([128, 64], U16, tag=f"i2{tb}")
                # ---- scores + stage-1 top8 (exact) ----
                for m in range(N_HEADS):
                    for half, (kp, st_, ix) in enumerate(
                        ((kp1_sb, s1t, i1), (kp2_sb, s2t, i2))
                    ):
                        ps = pss.tile([128, N_KEYS], F32, tag="pss")
                        nc.tensor.matmul(
                            out=ps[:],
                            lhsT=qt_all[:, 2 * m + half, tsl],
                            rhs=kp[:, m, :],
                            start=True, stop=True,
                        )
                        s_sb = scpool.tile([128, N_KEYS], F32, tag="s_sb")
                        nc.scalar.copy(out=s_sb[:], in_=ps[:])
                        nc.vector.max(out=st_[:, m * 8:(m + 1) * 8], in_=s_sb[:])
                        nc.vector.max_index(
                            out=ix[:, m * 8:(m + 1) * 8],
                            in_max=st_[:, m * 8:(m + 1) * 8],
                            in_values=s_sb[:],
                        )

                # ---- stage-2: 8x8 combo scores, mantissa-encoded top8 ----
                cs = st2.tile([128, 512], F32, tag=f"cs{tb}")
                for m in range(N_HEADS):
                    nc.vector.tensor_tensor(
                        out=cs[:, m * 64:(m + 1) * 64].rearrange(
                            "p (a b) -> p a b", a=8),
                        in0=s1t[:, m * 8:(m + 1) * 8].unsqueeze(2).to_broadcast(
                            [128, 8, 8]),
                        in1=s2t[:, m * 8:(m + 1) * 8].unsqueeze(1).to_broadcast(
                            [128, 8, 8]),
                        op=OP.add,
                    )
                v8 = st2.tile([128, 64], F32, tag=f"v8{tb}")
                n8 = st2.tile([128, 64], U16, tag=f"n8{tb}")
                for m in range(N_HEADS):
                    nc.vector.max(out=v8[:, m * 8:(m + 1) * 8],
                                  in_=cs[:, m * 64:(m + 1) * 64])
                    nc.vector.max_index(
                        out=n8[:, m * 8:(m + 1) * 8],
                        in_max=v8[:, m * 8:(m + 1) * 8],
                        in_values=cs[:, m * 64:(m + 1) * 64])
                k1 = st2.tile([128, 64], U16, tag=f"k1{tb}")
                nc.vector.tensor_scalar(
                    out=k1[:], in0=n8[:], scalar1=3, scalar2=None,
                    op0=OP.logical_shift_right)
                k2 = st2.tile([128, 64], U16, tag=f"k2{tb}")
                nc.vector.tensor_scalar(
                    out=k2[:], in0=n8[:], scalar1=7, scalar2=None,
                    op0=OP.bitwise_and)

                # resolve winners' sub-key ids: isel[p,m,j] = i[p,m,k1[p,m,j]]
                sels = []
                for kk, ix in ((k1, i1), (k2, i2)):
                    eq = eqs.tile([128, 512], U16, tag="eq")
                    nc.vector.tensor_tensor(
                        out=eq[:, :].rearrange("p (m j k) -> p m j k", m=8, j=8),
                        in0=kk[:, :].rearrange("p (m j) -> p m j", m=8)
                            .unsqueeze(3).to_broadcast([128, 8, 8, 8]),
                        in1=iota8[:, :].unsqueeze(1).unsqueeze(1)
                            .to_broadcast([128, 8, 8, 8]),
                        op=OP.is_equal)
                    prod = eqs.tile([128, 512], U16, tag="prod")
                    nc.vector.tensor_tensor(
                        out=prod[:, :].rearrange("p (m j k) -> p m j k", m=8, j=8),
                        in0=eq[:, :].rearrange("p (m j k) -> p m j k", m=8, j=8),
                        in1=ix[:, :].rearrange("p (m k) -> p m k", m=8)
                            .unsqueeze(2).to_broadcast([128, 8, 8, 8]),
                        op=OP.mult)
                    sel = st2.tile([128, 64], U16, tag=f"sel{len(sels)}{tb}")
                    with nc.allow_low_precision(
                            reason="one-hot uint16 sum, values <= 255"):
                        nc.vector.reduce_sum(
                            out=sel[:],
                            in_=prod[:, :].rearrange("p (mj k) -> p mj k", k=8),
                            axis=X)
                    sels.append(sel)
                idx16 = st2.tile([128, 64], U16, tag=f"idx16{tb}")
                nc.vector.tensor_scalar(
                    out=idx16[:], in0=sels[0][:], scalar1=256, scalar2=None,
                    op0=OP.mult)
                nc.vector.tensor_tensor(
                    out=idx16[:], in0=idx16[:], in1=sels[1][:], op=OP.add)
                ix32 = st2.tile([128, 64], I32, tag=f"idx32{tb}")
                nc.vector.tensor_copy(out=ix32[:], in_=idx16[:])
                idx32.append(ix32)
                v8s.append(v8)

            if stage == "routing":
                for tb in range(TB):
                    dbg = st2.tile([128, 64], F32, tag=f"dbg{tb}")
                    nc.vector.tensor_copy(out=dbg[:], in_=idx32[tb][:])
                    nc.sync.dma_start(
                        out=out_d[tb * 128:(tb + 1) * 128, 0:64], in_=dbg[:])
                    nc.sync.dma_start(
                        out=out_d[tb * 128:(tb + 1) * 128, 64:128],
                        in_=v8s[tb][:])

            # ---- softmax over each head's top-8 (on encoded scores) ----
            ws = []
            for tb in (() if stage == "routing" else range(TB)):
                v8 = v8s[tb]
                rmax = st2.tile([128, 8], F32, tag=f"rmax{tb}")
                nc.vector.reduce_max(
                    out=rmax[:], in_=v8[:, :].rearrange("p (m k) -> p m k", m=8),
                    axis=X)
                ex = st2.tile([128, 64], F32, tag=f"ex{tb}")
                nc.vector.tensor_tensor(
                    out=ex[:, :].rearrange("p (m k) -> p m k", m=8),
                    in0=v8[:, :].rearrange("p (m k) -> p m k", m=8),
                    in1=rmax[:, :].unsqueeze(2).to_broadcast([128, 8, 8]),
                    op=OP.subtract)
                nc.scalar.activation(
                    out=ex[:], in_=ex[:], func=mybir.ActivationFunctionType.Exp)
                rsum = st2.tile([128, 8], F32, tag=f"rsum{tb}")
                nc.vector.reduce_sum(
                    out=rsum[:], in_=ex[:, :].rearrange("p (m k) -> p m k", m=8),
                    axis=X)
                rinv = st2.tile([128, 8], F32, tag=f"rinv{tb}")
                nc.vector.reciprocal(out=rinv[:], in_=rsum[:])
                w8 = st2.tile([128, 64], F32, tag=f"w8{tb}")
                nc.vector.tensor_tensor(
                    out=w8[:, :].rearrange("p (m k) -> p m k", m=8),
                    in0=ex[:, :].rearrange("p (m k) -> p m k", m=8),
                    in1=rinv[:, :].unsqueeze(2).to_broadcast([128, 8, 8]),
                    op=OP.mult)
                ws.append(w8)

            # ---- fused main loop: gather both rows, inner, combine ----
            # Token-blocks are interleaved at group granularity so both
            # PSUM accumulation chains (PE) and gather streams (GpSimd)
            # stay busy concurrently.
            tbs = () if stage in ("routing",) else tuple(range(TB))
            inner = {}
            va = {}
            pacc = {}
            pages = {}
            for tb in tbs:
                inner[tb] = st2.tile([128, 64], F32, tag=f"inner{tb}", name=f"inner{tb}")
                va[tb] = st2.tile([128, 64], F32, tag=f"va{tb}", name=f"va{tb}")
                pacc[tb] = paccp.tile([128, D], F32, tag=f"pacc{tb}", name=f"pacc{tb}")
            for grp in range(8):
                for tb in tbs:
                    for sidx in range(8):
                        col = grp * 8 + sidx
                        page = wbp.tile([128, 2 * D], BF16, tag="wbpage",
                                        name=f"pg{tb}_{col}")
                        pages[(tb, col)] = page
                        nc.gpsimd.indirect_dma_start(
                            out=page[:], out_offset=None,
                            in_=wb_d[:, :],
                            in_offset=IndirectOffsetOnAxis(
                                ap=idx32[tb][:, col:col + 1], axis=0),
                        )
                        scr = ttrp.tile([128, D], BF16, tag="ttr_scr")
                        nc.vector.tensor_tensor(
                            out=scr[:], in0=page[:, 0:D], in1=xtok_bf[tb][:],
                            op=OP.mult)
                        scr2 = ttrp.tile([128, D], BF16, tag="ttr_scr2")
                        nc.scalar.activation(
                            out=scr2[:], in_=scr[:],
                            func=mybir.ActivationFunctionType.Copy,
                            accum_out=inner[tb][:, col:col + 1])
                    gs = slice(grp * 8, (grp + 1) * 8)
                    rl8 = st2.tile([128, 8], F32, tag=f"rl{tb}")
                    nc.scalar.activation(
                        out=rl8[:], in_=inner[tb][:, gs],
                        func=mybir.ActivationFunctionType.Relu)
                    if stage == "wdown":
                        nc.vector.tensor_copy(out=va[tb][:, gs], in_=rl8[:])
                    else:
                        nc.vector.tensor_tensor(
                            out=va[tb][:, gs], in0=rl8[:], in1=ws[tb][:, gs],
                            op=OP.mult)
                    for sidx in range(8):
                        col = grp * 8 + sidx
                        diag = dgp.tile([128, 128], BF16, tag="diag")
                        nc.vector.tensor_tensor(
                            out=diag[:],
                            in0=va[tb][:, col:col + 1].to_broadcast([128, 128]),
                            in1=id01_sb[:], op=OP.mult)
                        nc.tensor.matmul(
                            out=pacc[tb][:], lhsT=diag[:],
                            rhs=pages[(tb, col)][:, D:2 * D],
                            start=(col == 0), stop=(col == 63))
            for tb in tbs:
                acc_sb = accp.tile([128, D], F32, tag=f"acc{tb}")
                nc.vector.tensor_copy(out=acc_sb[:], in_=pacc[tb][:])
                if stage == "wdown":
                    nc.sync.dma_start(
                        out=out_d[tb * 128:(tb + 1) * 128, 0:64],
                        in_=inner[tb][:])
                else:
                    nc.sync.dma_start(
                        out=out_d[tb * 128:(tb + 1) * 128, :], in_=acc_sb[:])

    nc.compile()
    return nc


_NC_CACHE = None


def _get_nc():
    global _NC_CACHE
    if _NC_CACHE is None:
        _NC_CACHE = build_nc()
    return _NC_CACHE


def _prep_in_maps(inputs):
    q = np.ascontiguousarray(np.asarray(inputs["queries"], dtype=np.float32))
    Wq = np.ascontiguousarray(np.asarray(inputs["Wq"], dtype=np.float32))
    bq = np.asarray(inputs["bq"], dtype=np.float32)
    keys = np.asarray(inputs["keys"], dtype=np.float32)
    wd = np.asarray(inputs["w_down"], dtype=np.float32)
    wu = np.asarray(inputs["w_up"], dtype=np.float32)
    import ml_dtypes
    wb = np.ascontiguousarray(
        np.concatenate([wd, wu], axis=1).astype(ml_dtypes.bfloat16))
    id01 = np.eye(128, dtype=np.float32).astype(ml_dtypes.bfloat16)

    x = q.reshape(B, D)
    # bqp[p, mj] = bq[mj*64 + p]
    bqp = np.ascontiguousarray(bq.reshape(16, HALF).T)
    # kp{1,2}[c, m, n] = keys[m, half, n, c]
    kp1 = np.ascontiguousarray(keys[:, 0].transpose(2, 0, 1))
    kp2 = np.ascontiguousarray(keys[:, 1].transpose(2, 0, 1))

    in_maps = []
    for c in range(N_CORES):
        xc = x[c * BC:(c + 1) * BC]
        in_maps.append({
            "xtok": np.ascontiguousarray(xc),
            "xt": np.ascontiguousarray(xc.T),
            "wq": Wq,
            "bqp": bqp,
            "kp1": kp1,
            "kp2": kp2,
            "wb": wb,
            "id01": id01,
        })
    return in_maps


def run(inputs, trace=False):
    """Run on 8 NeuronCores; returns (out [2,1024,512], BassKernelResults)."""
    nc = _get_nc()
    in_maps = _prep_in_maps(inputs)
    res = run_bass_kernel_spmd(
        nc, in_maps, core_ids=list(range(N_CORES)), trace=trace)
    out = np.concatenate(
        [res.results[c]["out"] for c in range(N_CORES)], axis=0)
    return out.reshape(2, 1024, D), res


def kernel(**inputs) -> np.ndarray:
    out, _ = run(inputs, trace=False)
    return out
